# revision 26
# baseline (speedup 1.0000x reference)
"""Trainium2 Bass kernel for nn_AttentionSeqToMasked (dense transformer attention).

Full-input contract: kernel(**inputs) takes the unsharded numpy inputs and
returns the full [B, SQ, H*D_V] float32 output.

Sharding (8 cores): data parallel over batch (B=4 -> 2 cores per batch) x
tensor parallel over heads (16 heads -> 8 per core). Each core computes
attention for one (batch, head-half) pair; host gathers the slices.

Per-core dataflow (all matmuls bf16, fp32 PSUM accumulation):
  - Host pre-transposes activations to X^T [D_PRE, S] bf16; inputs stream in
    512-column chunks, ordered so the first scores tile unblocks ~15us in
    (wq/wk + c0 chunks first, xv interleaved with the xk/xq tails).
  - Projections compute q^T/k^T = W^T @ X^T (head-dim on partitions); v in
    natural [k, d_v] layout with a ones-column appended via the weight
    matrix (zero weight column + bias 1.0) for the softmax denominator.
  - Scores are computed transposed: scoresT[k, q] = kT.T @ qT; the two heads
    of a pair are K=64 matmuls at PE row groups 0/64 and run concurrently
    (~174ns for both vs ~260ns each at K=128).
  - The key-mask folds into exp as a per-partition bias; 1/sqrt(d) folds
    into the exp scale. exp runs on ScalarE from PSUM (its ~1.0-1.2us per
    [128,1024] tile is the pacing wall), bf16 out.
  - AV contracts exp(scores)T with [v | ones]: psum row 64 is the softmax
    denominator, free. A final PE transpose returns [q, d_v+1] tiles;
    VectorE multiplies by the reciprocal denominator; result DMAs to DRAM.

Schedule: 128 blocks of 2 exp tiles. Per block the PE runs 2 score pairs
(lookahead 2, lockstepped to exp via the 2 sc psum bufs), the previous
block's 4 AV matmuls (in-order queue, gated on the v-projection filler
positions, catch-up cap 3), one projection-filler unit (3/block for blocks
0-14 to finish v for pairs 0/1 early - the 11 ex-tile bufs bound the AV
deferral - then 1/block), and any deferred epilogue (avt copies fire at
AV-stop; transposes/normalize run a block later so PE's in-order queue
never stalls on VectorE). Emission order IS per-engine program order: every
tile write is emitted before its readers; the gate arithmetic in the filler
lattice is exact-fit and asserted at build time.

Known-dead optimization avenues (measured on hardware, do not retry):
  - fp8e4 anywhere in the softmax value path (ex, v, q/k projections):
    concentrated-softmax rows amplify the 2.6% quantization noise ~13
    sigma; measured rel err 5.1e-2 vs the 2e-2 budget (bf16 gives 7.2e-3).
    fp8 DoubleRow AV itself works and is 2x (validated in isolation).
  - Split-K row-tiled concurrent AV (K=64 halves at row groups 0/64 into
    the two av psum banks): correct and fast in a microbenchmark but the
    full kernel dies at NEFF execution (NRT INTERNAL error).
  - Strided-AP reciprocal over PSUM (tp[:, 64:260:65]): +60us (PSUM 8B
    cachelines make non-contiguous APs very slow).
"""

import os
from contextlib import ExitStack

import numpy as np
import ml_dtypes

import concourse.bass as bass
import concourse.bacc as bacc
import concourse.mybir as mybir
import concourse.tile as tile
from concourse.bass_utils import run_bass_kernel_spmd
from concourse.masks import make_identity

# Problem shape (hardcoded per contract)
B, SQ, SK = 4, 2048, 2048
D_PRE = 1024
H, D_QK, D_V = 16, 64, 64
N_CORES = 8
HALF = (H // 2) * D_QK  # 512 columns of the projection handled per core
N_PAIRS = 4  # head pairs per core
S_CHUNK = 512  # moving free-dim per matmul
N_DT = D_PRE // 128  # d_pre tiles of 128
N_KT = SK // 128  # key tiles of 128
N_KTP = N_KT // 2  # key-tile pairs (DoubleRow granularity)
N_QC = SQ // S_CHUNK  # query chunks of 512
MASK_NEG = -30000.0
EX_OFF = 0.0
V_STRIDE = 130  # v pair-tile inner stride (65 cols per head)

F32 = mybir.dt.float32
BF16 = mybir.dt.bfloat16
FP8 = mybir.dt.float8e4
BF16_NP = np.dtype(ml_dtypes.bfloat16)


_COMPILED = None


def _build_program():
    nc = bacc.Bacc("TRN2", target_bir_lowering=False, debug=False)

    # DRAM I/O (names are the in_map keys)
    xq = nc.dram_tensor("xq", [D_PRE, SQ], BF16, kind="ExternalInput").ap()
    xk = nc.dram_tensor("xk", [D_PRE, SK], BF16, kind="ExternalInput").ap()
    xv = nc.dram_tensor("xv", [D_PRE, SK], BF16, kind="ExternalInput").ap()
    wq = nc.dram_tensor("wq", [D_PRE, HALF], BF16, kind="ExternalInput").ap()
    wk = nc.dram_tensor("wk", [D_PRE, HALF], BF16, kind="ExternalInput").ap()
    # v weights with a zero column appended per head (ones column generator)
    wv = nc.dram_tensor("wv", [D_PRE, N_PAIRS * 130], BF16, kind="ExternalInput").ap()
    bq = nc.dram_tensor("bq", [128, N_PAIRS], F32, kind="ExternalInput").ap()
    bk = nc.dram_tensor("bk", [128, N_PAIRS], F32, kind="ExternalInput").ap()
    bv = nc.dram_tensor("bv", [128, N_PAIRS * 130], F32, kind="ExternalInput").ap()
    mb = nc.dram_tensor("mb", [128, N_KT], F32, kind="ExternalInput").ap()
    out = nc.dram_tensor("out", [SQ, HALF], F32, kind="ExternalOutput").ap()

    with tile.TileContext(nc) as tc:
        _emit(tc, xq, xk, xv, wq, wk, wv, bq, bk, bv, mb, out)

    nc.compile()
    return nc


def _emit(tc, xq, xk, xv, wq, wk, wv, bq, bk, bv, mb, out):
    nc = tc.nc

    with ExitStack() as ctx:
        # ---- pools ----
        xp = ctx.enter_context(tc.tile_pool(name="x", bufs=3 * N_DT * N_QC))
        wp = ctx.enter_context(tc.tile_pool(name="w", bufs=1))
        cp = ctx.enter_context(tc.tile_pool(name="const", bufs=1))
        qkvp = ctx.enter_context(tc.tile_pool(name="qkv", bufs=1))
        expp = ctx.enter_context(tc.tile_pool(name="exp", bufs=11))
        avtp = ctx.enter_context(tc.tile_pool(name="avt", bufs=2))
        stgp = ctx.enter_context(tc.tile_pool(name="stg", bufs=4))
        rp = ctx.enter_context(tc.tile_pool(name="recip", bufs=8))

        proj_ps = ctx.enter_context(tc.tile_pool(name="proj_ps", bufs=1, space="PSUM"))
        sc_ps = ctx.enter_context(tc.tile_pool(name="sc_ps", bufs=2, space="PSUM"))
        av_ps = ctx.enter_context(tc.tile_pool(name="av_ps", bufs=2, space="PSUM"))
        tp_ps = ctx.enter_context(tc.tile_pool(name="tp_ps", bufs=1, space="PSUM"))

        # ---- constants (tiny, load first) ----
        ident = cp.tile([128, 128], F32, name="ident")
        make_identity(nc, ident)
        mb_sb = cp.tile([128, N_KT], F32, name="mb_sb")
        nc.sync.dma_start(mb_sb, mb)
        bq_sb = cp.tile([128, N_PAIRS], F32, name="bq_sb")
        nc.sync.dma_start(bq_sb, bq)
        bk_sb = cp.tile([128, N_PAIRS], F32, name="bk_sb")
        nc.sync.dma_start(bk_sb, bk)
        bv_sb = cp.tile([128, N_PAIRS * 130], F32, name="bv_sb")
        nc.sync.dma_start(bv_sb, bv)

        # ---- streamed loads in qc-chunks, ordered so the attention stream
        # unblocks ASAP: wq,wk + c0 of xq,xk first; xk tail (kt sweep), then
        # xq tail (later qc), then xv + wv (AV is deferred past these) ----
        def load_w(wap, pfx, width):
            ts = []
            for dt_i in range(N_DT):
                t = wp.tile([128, width], BF16, name=f"{pfx}{dt_i}", tag=f"{pfx}{dt_i}")
                nc.sync.dma_start(t, wap[dt_i * 128 : (dt_i + 1) * 128, :])
                ts.append(t)
            return ts

        x_tiles = {}  # (pfx, dt, c) -> [128, 512] bf16

        def load_x_chunk(xap, pfx, c):
            for dt_i in range(N_DT):
                t = xp.tile([128, S_CHUNK], BF16, name=f"{pfx}{dt_i}_{c}", tag="x")
                nc.sync.dma_start(
                    t, xap[dt_i * 128 : (dt_i + 1) * 128, c * S_CHUNK : (c + 1) * S_CHUNK]
                )
                x_tiles[(pfx, dt_i, c)] = t

        wq_sb = load_w(wq, "wq", HALF)
        wk_sb = load_w(wk, "wk", HALF)
        load_x_chunk(xq, "q", 0)
        load_x_chunk(xk, "k", 0)
        load_x_chunk(xk, "k", 1)
        wv_sb = load_w(wv, "wv", N_PAIRS * 130)
        load_x_chunk(xv, "v", 0)
        load_x_chunk(xk, "k", 2)
        load_x_chunk(xv, "v", 1)
        load_x_chunk(xk, "k", 3)
        load_x_chunk(xq, "q", 1)
        load_x_chunk(xv, "v", 2)
        load_x_chunk(xq, "q", 2)
        load_x_chunk(xv, "v", 3)
        load_x_chunk(xq, "q", 3)

        v_tiles = {}  # (pair, ktp) -> [128, 2, V_STRIDE] fp8 tile
        qkT = {}  # (pfx, pair) -> [128, SQ] bf16 tile

        def qk_tile(pfx, pair):
            if (pfx, pair) not in qkT:
                qkT[(pfx, pair)] = qkvp.tile(
                    [128, SQ], BF16, name=f"{pfx}T{pair}", tag=f"{pfx}T", bufs=2
                )
            return qkT[(pfx, pair)]

        def v_tile(pair, ktp):
            if (pair, ktp) not in v_tiles:
                v_tiles[(pair, ktp)] = qkvp.tile(
                    [128, 2 * V_STRIDE], BF16, name=f"v{pair}_{ktp}", tag="v",
                    bufs=N_PAIRS * N_KTP,
                )
            return v_tiles[(pair, ktp)]

        proj_ps_open = {}

        def emit_qk_half(pair, pfx, qc, half):
            # half a [128, 512] projection chunk: 4 accumulating MMs; the
            # second half adds the bias copy. Halves of one chunk must be
            # popped back-to-back (proj_ps bufs=1).
            dst = qk_tile(pfx, pair)
            w_sb = wq_sb if pfx == "q" else wk_sb
            b_sb = bq_sb if pfx == "q" else bk_sb
            xpfx = "q" if pfx == "q" else "k"
            key = (pair, pfx, qc)
            if half == 1:
                ps = proj_ps_open.pop(key)
            else:
                ps = proj_ps.tile(
                    [128, S_CHUNK], F32, name=f"{pfx}ps{pair}_{qc}", tag="proj"
                )
            for dt_i in range(half * 4, half * 4 + 4):
                nc.tensor.matmul(
                    ps,
                    lhsT=w_sb[dt_i][:, pair * 128 : (pair + 1) * 128],
                    rhs=x_tiles[(xpfx, dt_i, qc)],
                    start=(dt_i == 0),
                    stop=(dt_i == N_DT - 1),
                )
            if half == 0:
                proj_ps_open[key] = ps
            else:
                nc.vector.tensor_scalar_add(
                    dst[:, qc * S_CHUNK : (qc + 1) * S_CHUNK],
                    ps,
                    b_sb[:, pair : pair + 1],
                )

        def emit_v_half(g, st, half):
            # v projection for pairs (2g, 2g+1), one key tile st, one half of
            # the contraction; half 1 finishes with fp8 writes into the
            # DoubleRow pair-tiles.
            key = ("v", g, st)
            if half == 1:
                ps = proj_ps_open.pop(key)
            else:
                ps = proj_ps.tile([128, S_CHUNK], F32, name=f"vps{g}_{st}", tag="proj")
            qc = st // 4  # key tile st lives in x chunk st//4
            col = (st % 4) * 128
            for dt_i in range(half * 4, half * 4 + 4):
                nc.tensor.matmul(
                    ps[:, 0:260],
                    lhsT=x_tiles[("v", dt_i, qc)][:, col : col + 128],
                    rhs=wv_sb[dt_i][:, g * 260 : (g + 1) * 260],
                    start=(dt_i == 0),
                    stop=(dt_i == N_DT - 1),
                )
            if half == 0:
                proj_ps_open[key] = ps
                return
            ktp, j = st // 2, st % 2
            for p_i in range(2):
                pair = 2 * g + p_i
                vt = v_tile(pair, ktp)
                nc.vector.tensor_add(
                    vt[:, j * 130 : j * 130 + 130],
                    ps[:, p_i * 130 : p_i * 130 + 130],
                    bv_sb[:, pair * 130 : pair * 130 + 130],
                )

        # filler queue: 4-MM units popped POPS_PER_BLOCK per block. Emission
        # order IS program order per engine — every tile write must be
        # emitted before its readers. Gates below are derived from unit
        # positions. Halves of a chunk stay adjacent (shared open psum).
        filler = []

        def _qk(pair, pfx, qc, half):
            return lambda: emit_qk_half(pair, pfx, qc, half)

        def _v(g, st, half):
            return lambda: emit_v_half(g, st, half)

        v_gate = {}  # (g, st) -> first block the v pair-half is available

        # pop schedule: 3/block for blocks 0-14 (v g0 + p0 tail), then 1/block
        def pops_at(b):
            return 3 if b < 15 else 1

        def pop_block(pos):
            if pos < 45:
                return pos // 3
            return 15 + (pos - 45)

        def add_v(g, st, half):
            filler.append(_v(g, st, half))
            if half == 1:
                v_gate[(g, st)] = pop_block(len(filler) - 1) + 1

        # interleaved head: p0 k-tail (kt sweep gates) and v g0 (AV gates),
        # then p0 q-tail, p1, v g1 interleaved with p2, p3. Popped 3/block
        # for blocks 0-14 (pos 0-44), then 1/block (pos 45+k -> block 15+k).
        qk_gate = {}

        def add_qk(pair, pfx, c, half):
            filler.append(_qk(pair, pfx, c, half))
            if half == 1:
                qk_gate[(pair, pfx, c)] = pop_block(len(filler) - 1) + 1

        add_qk(0, "k", 1, 0); add_qk(0, "k", 1, 1)       # pos 0-1
        add_v(0, 0, 0); add_v(0, 0, 1)                   # 2-3
        add_qk(0, "k", 2, 0); add_qk(0, "k", 2, 1)       # 4-5
        add_v(0, 1, 0); add_v(0, 1, 1)                   # 6-7
        add_qk(0, "k", 3, 0); add_qk(0, "k", 3, 1)       # 8-9
        for st in (2, 3):
            add_v(0, st, 0); add_v(0, st, 1)             # 10-13
        add_qk(0, "q", 1, 0); add_qk(0, "q", 1, 1)       # 14-15
        for st in (4, 5, 6, 7):
            add_v(0, st, 0); add_v(0, st, 1)             # 16-23
        add_qk(0, "q", 2, 0); add_qk(0, "q", 2, 1)       # 24-25
        for st in (8, 9, 10, 11):
            add_v(0, st, 0); add_v(0, st, 1)             # 26-33
        add_qk(0, "q", 3, 0); add_qk(0, "q", 3, 1)       # 34-35
        for st in (12, 13, 14, 15):
            add_v(0, st, 0); add_v(0, st, 1)             # 36-43
        add_qk(1, "q", 0, 0)                             # 44 (front ends)
        add_qk(1, "q", 0, 1)                             # 45
        add_qk(1, "k", 0, 0); add_qk(1, "k", 0, 1)       # 46-47
        for pfx in ("q", "k"):
            for c in range(1, N_QC):
                add_qk(1, pfx, c, 0); add_qk(1, pfx, c, 1)  # 48-59
        for st in range(14):
            add_v(1, st, 0); add_v(1, st, 1)             # 60-87
        add_qk(2, "q", 0, 0); add_qk(2, "q", 0, 1)       # 88-89
        add_qk(2, "k", 0, 0); add_qk(2, "k", 0, 1)       # 90-91
        add_qk(2, "k", 1, 0); add_qk(2, "k", 1, 1)       # 92-93
        add_qk(2, "k", 2, 0); add_qk(2, "k", 2, 1)       # 94-95
        add_qk(2, "k", 3, 0); add_qk(2, "k", 3, 1)       # 96-97
        add_qk(2, "q", 1, 0); add_qk(2, "q", 1, 1)       # 98-99
        add_v(1, 14, 0); add_v(1, 14, 1)                 # 100-101
        add_v(1, 15, 0); add_v(1, 15, 1)                 # 102-103
        add_qk(2, "q", 2, 0); add_qk(2, "q", 2, 1)       # 104-105
        add_qk(2, "q", 3, 0); add_qk(2, "q", 3, 1)       # 106-107
        for pfx in ("q", "k"):
            for c in range(N_QC):
                add_qk(3, pfx, c, 0); add_qk(3, pfx, c, 1)  # 108-123

        def pop_filler(n):
            for _ in range(n):
                if filler:
                    filler.pop(0)()

        # warm the ACT exp table during the input DMA
        warm = cp.tile([128, 1], F32, name="actwarm")
        nc.scalar.activation(warm, bq_sb[:, 0:1],
                             mybir.ActivationFunctionType.Exp, bias=0.0, scale=0.0)

        # prologue: only what gates the first scores tiles
        emit_qk_half(0, "q", 0, 0)
        emit_qk_half(0, "q", 0, 1)
        emit_qk_half(0, "k", 0, 0)
        emit_qk_half(0, "k", 0, 1)

        # ---- software-pipelined attention stream over (pair, qc, kt) ----
        iters = [
            (pair, qc, kt)
            for pair in range(N_PAIRS)
            for qc in range(N_QC)
            for kt in range(N_KT)
        ]
        n_it = len(iters)
        sc_map = {}
        av_map = {}
        ex_map = {}  # block -> expair tile [128, 2, 1024] fp8

        def emit_scores(i, blk=None):
            pair, qc, kt = iters[i]
            if blk is not None and pair > 0:
                assert qk_gate[(pair, "q", qc)] <= blk, (i, "q", qc)
                assert qk_gate[(pair, "k", kt // 4)] <= blk, (i, "k", kt // 4)
            qT = qk_tile("q", pair)
            kT = qk_tile("k", pair)
            sc = sc_ps.tile([128, 1024], F32, name=f"sc{pair}_{qc}_{kt}", tag="sc")
            # scoresT for heads A and B, row-tiled concurrently (K=64 groups)
            nc.tensor.matmul(
                sc[:, 0:512],
                lhsT=kT[0:64, kt * 128 : (kt + 1) * 128],
                rhs=qT[0:64, qc * S_CHUNK : (qc + 1) * S_CHUNK],
                start=True,
                stop=True,
            )
            nc.tensor.matmul(
                sc[:, 512:1024],
                lhsT=kT[64:128, kt * 128 : (kt + 1) * 128],
                rhs=qT[64:128, qc * S_CHUNK : (qc + 1) * S_CHUNK],
                start=True,
                stop=True,
            )
            sc_map[i] = sc

        def emit_exp(i, ex):
            pair, qc, kt = iters[i]
            nc.scalar.activation(
                ex[:, (kt % 2) * 1024 : (kt % 2) * 1024 + 1024],
                sc_map.pop(i),
                mybir.ActivationFunctionType.Exp,
                bias=mb_sb[:, kt : kt + 1],
                scale=0.125,
            )

        epi_pending = []

        def emit_epilogue(pair, qc, avts):
            # transpose back to [q, d_v], normalize, store
            stgs = [
                stgp.tile([128, 128], F32, name=f"st{pair}_{qc}_{u}", tag="stg")
                for u in range(4)
            ]
            for h_i, avt in enumerate(avts):
                tp = tp_ps.tile([128, 260], F32, name=f"tp{pair}_{qc}_{h_i}", tag="tp")
                for u in range(4):
                    nc.tensor.transpose(
                        tp[:, u * 65 : u * 65 + 65],
                        avt[:, u * 128 : (u + 1) * 128],
                        ident[0:65, 0:65],
                    )
                for u in range(4):
                    rc = rp.tile([128, 1], F32, name=f"rc{pair}_{qc}_{h_i}_{u}", tag="rc")
                    nc.vector.reciprocal(rc, tp[:, u * 65 + 64 : u * 65 + 65])
                    nc.vector.tensor_scalar_mul(
                        stgs[u][:, h_i * 64 : (h_i + 1) * 64],
                        tp[:, u * 65 : u * 65 + 64],
                        rc,
                    )
            for u in range(4):
                qt = qc * 4 + u
                nc.sync.dma_start(
                    out[qt * 128 : (qt + 1) * 128, pair * 128 : (pair + 1) * 128],
                    stgs[u],
                )

        av_active = []  # (pair, qc) sets with an open av psum pair (max 2)

        def emit_av_block(b):
            # DoubleRow AV for block b = iters (2b, 2b+1): 2 MMs, both heads
            pair, qc, kt = iters[2 * b]
            ktp = kt // 2
            ex = ex_map.pop(b)
            if ktp == 0:
                av_map[(pair, qc)] = (
                    av_ps.tile([65, S_CHUNK], F32, name=f"ava{pair}_{qc}", tag="av"),
                    av_ps.tile([65, S_CHUNK], F32, name=f"avb{pair}_{qc}", tag="av"),
                )
                av_active.append((pair, qc))
            av_a, av_b = av_map[(pair, qc)]
            assert (pair, ktp) in v_tiles, (pair, ktp)
            vt = v_tiles[(pair, ktp)]
            for j in range(2):
                st_ = ktp == 0 and j == 0
                sp_ = ktp == N_KTP - 1 and j == 1
                vo, eo = j * 130, j * 1024
                nc.tensor.matmul(
                    av_a, lhsT=vt[:, vo : vo + 65],
                    rhs=ex[:, eo : eo + 512],
                    start=st_, stop=sp_,
                )
                nc.tensor.matmul(
                    av_b, lhsT=vt[:, vo + 65 : vo + 130],
                    rhs=ex[:, eo + 512 : eo + 1024],
                    start=st_, stop=sp_,
                )
            if ktp == N_KTP - 1:
                av_a, av_b = av_map.pop((pair, qc))
                avts = []
                for h_i, av in enumerate((av_a, av_b)):
                    avt = avtp.tile(
                        [65, S_CHUNK], F32, name=f"avt{pair}_{qc}_{h_i}", tag="avt"
                    )
                    nc.vector.tensor_copy(avt, av)
                    avts.append(avt)
                epi_pending.append((pair, qc, avts))
                av_active.remove((pair, qc))

        def av_legal(b_pend, b_now):
            # emission-order gates: v pair-tile written, expair written (a
            # previous block), and at most 2 av psum sets in flight
            pair, qc, kt = iters[2 * b_pend]
            ktp = kt // 2
            if v_gate[(pair // 2, 2 * ktp + 1)] > b_now:
                return False
            if b_pend >= b_now:
                return False
            if ktp == 0 and len(av_active) >= 2 and (pair, qc) not in av_active:
                return False
            return True

        # stream prologue: scores for iters 0,1 (lookahead is 2)
        emit_scores(0)
        emit_scores(1)

        n_blocks = n_it // 2
        av_pending = list(range(n_blocks))  # in-order AV emission queue
        for b in range(n_blocks):
            i0, i1 = 2 * b, 2 * b + 1
            ex = expp.tile([128, 2048], BF16, name=f"ex{b}", tag="ex")
            ex_map[b] = ex
            emit_exp(i0, ex)
            emit_exp(i1, ex)
            if i0 + 2 < n_it:
                emit_scores(i0 + 2, blk=b)
                emit_scores(i0 + 3, blk=b)
            while epi_pending:
                emit_epilogue(*epi_pending.pop(0))
            budget = 3
            while av_pending and budget and av_legal(av_pending[0], b):
                emit_av_block(av_pending.pop(0))
                budget -= 1
            pop_filler(pops_at(b))
        for b_pend in av_pending:
            emit_av_block(b_pend)
        while epi_pending:
            emit_epilogue(*epi_pending.pop(0))

        assert not filler, f"{len(filler)} filler chunks left unscheduled"
        assert not ex_map and not av_map and not sc_map, (
            len(ex_map), len(av_map), len(sc_map))
        assert not proj_ps_open


def _prep_core_inputs(pre_qs, pre_ks, pre_vs, k_mask, q_w, q_b, k_w, k_b, v_w, v_b, core):
    b = core // 2
    hh = core % 2
    cols = slice(HALF * hh, HALF * (hh + 1))

    xq = np.ascontiguousarray(pre_qs[b].T).astype(BF16_NP)
    xk = np.ascontiguousarray(pre_ks[b].T).astype(BF16_NP)
    xv = np.ascontiguousarray(pre_vs[b].T).astype(BF16_NP)
    wq = np.ascontiguousarray(q_w[:, cols]).astype(BF16_NP)
    wk = np.ascontiguousarray(k_w[:, cols]).astype(BF16_NP)

    wv_core = v_w[:, cols].astype(np.float32)
    wv = np.zeros((D_PRE, N_PAIRS * 130), dtype=np.float32)
    bv_core = v_b[cols].astype(np.float32)
    bv_ext = np.zeros(N_PAIRS * 130, dtype=np.float32)
    for p in range(N_PAIRS):
        wv[:, p * 130 : p * 130 + 64] = wv_core[:, p * 128 : p * 128 + 64]
        wv[:, p * 130 + 65 : p * 130 + 129] = wv_core[:, p * 128 + 64 : p * 128 + 128]
        bv_ext[p * 130 : p * 130 + 64] = bv_core[p * 128 : p * 128 + 64]
        bv_ext[p * 130 + 64] = 1.0
        bv_ext[p * 130 + 65 : p * 130 + 129] = bv_core[p * 128 + 64 : p * 128 + 128]
        bv_ext[p * 130 + 129] = 1.0

    bq = np.ascontiguousarray(q_b[cols].astype(np.float32).reshape(N_PAIRS, 128).T)
    bk = np.ascontiguousarray(k_b[cols].astype(np.float32).reshape(N_PAIRS, 128).T)
    bv_full = np.ascontiguousarray(np.tile(bv_ext[None, :], (128, 1)))

    # mask True -> 0.0, False -> MASK_NEG; plus the fp8-range exp offset
    mbias = np.where(k_mask[b], 0.0, MASK_NEG).astype(np.float32) + EX_OFF
    mb = np.ascontiguousarray(mbias.reshape(N_KT, 128).T)

    return {
        "xq": xq,
        "xk": xk,
        "xv": xv,
        "wq": wq,
        "wk": wk,
        "wv": wv.astype(BF16_NP),
        "bq": bq,
        "bk": bk,
        "bv": bv_full,
        "mb": mb,
    }


def kernel(pre_qs, pre_ks, pre_vs, k_mask, q_w, q_b, k_w, k_b, v_w, v_b):
    global _COMPILED
    args = (pre_qs, pre_ks, pre_vs, k_mask, q_w, q_b, k_w, k_b, v_w, v_b)
    args = tuple(np.asarray(a) for a in args)

    if _COMPILED is None:
        _COMPILED = _build_program()
    nc = _COMPILED

    in_maps = [_prep_core_inputs(*args, core=c) for c in range(N_CORES)]

    trace = bool(int(os.environ.get("BASS_KERNEL_TRACE", "0")))
    res = run_bass_kernel_spmd(
        nc,
        in_maps,
        core_ids=list(range(N_CORES)),
        trace=trace,
    )
    if trace:
        kernel.last_results = res

    out = np.empty((B, SQ, H * D_V), dtype=np.float32)
    for c in range(N_CORES):
        b = c // 2
        hh = c % 2
        out[b, :, HALF * hh : HALF * (hh + 1)] = res.results[c]["out"]
    return out


# revision 27
# speedup vs baseline: 1.0208x; 1.0208x over previous
"""Trainium2 Bass kernel for nn_AttentionSeqToMasked (dense transformer attention).

Full-input contract: kernel(**inputs) takes the unsharded numpy inputs and
returns the full [B, SQ, H*D_V] float32 output.

Sharding (8 cores): data parallel over batch (B=4 -> 2 cores per batch) x
tensor parallel over heads (16 heads -> 8 per core). Each core computes
attention for one (batch, head-half) pair; host gathers the slices.

Per-core dataflow (all matmuls bf16, fp32 PSUM accumulation):
  - Host pre-transposes activations to X^T [D_PRE, S] bf16; inputs stream in
    512-column chunks, ordered so the first scores tile unblocks ~15us in
    (wq/wk + c0 chunks first, xv interleaved with the xk/xq tails).
  - Projections compute q^T/k^T = W^T @ X^T (head-dim on partitions); v in
    natural [k, d_v] layout with a ones-column appended via the weight
    matrix (zero weight column + bias 1.0) for the softmax denominator.
  - Scores are computed transposed: scoresT[k, q] = kT.T @ qT; the two heads
    of a pair are K=64 matmuls at PE row groups 0/64 and run concurrently
    (~174ns for both vs ~260ns each at K=128).
  - The key-mask folds into exp as a per-partition bias; 1/sqrt(d) folds
    into the exp scale. exp runs on ScalarE from PSUM (its ~1.0-1.2us per
    [128,1024] tile is the pacing wall), bf16 out.
  - AV contracts exp(scores)T with [v | ones]: psum row 64 is the softmax
    denominator, free. A final PE transpose returns [q, d_v+1] tiles;
    VectorE multiplies by the reciprocal denominator; result DMAs to DRAM.

Schedule: 128 blocks of 2 exp tiles. Per block the PE runs 2 score pairs
(lookahead 2, lockstepped to exp via the 2 sc psum bufs), the previous
block's 4 AV matmuls (in-order queue, gated on the v-projection filler
positions, catch-up cap 3), one projection-filler unit (3/block for blocks
0-14 to finish v for pairs 0/1 early - the 11 ex-tile bufs bound the AV
deferral - then 1/block), and any deferred epilogue (avt copies fire at
AV-stop; transposes/normalize run a block later so PE's in-order queue
never stalls on VectorE). Emission order IS per-engine program order: every
tile write is emitted before its readers; the gate arithmetic in the filler
lattice is exact-fit and asserted at build time.

Known-dead optimization avenues (measured on hardware, do not retry):
  - fp8e4 anywhere in the softmax value path (ex, v, q/k projections):
    concentrated-softmax rows amplify the 2.6% quantization noise ~13
    sigma; measured rel err 5.1e-2 vs the 2e-2 budget (bf16 gives 7.2e-3).
    fp8 DoubleRow AV itself works and is 2x (validated in isolation).
  - Split-K row-tiled concurrent AV (K=64 halves at row groups 0/64 into
    the two av psum banks): correct and fast in a microbenchmark but the
    full kernel dies at NEFF execution (NRT INTERNAL error).
  - Strided-AP reciprocal over PSUM (tp[:, 64:260:65]): +60us (PSUM 8B
    cachelines make non-contiguous APs very slow).
"""

import os
from contextlib import ExitStack

import numpy as np
import ml_dtypes

import concourse.bass as bass
import concourse.bacc as bacc
import concourse.mybir as mybir
import concourse.tile as tile
from concourse.bass_utils import run_bass_kernel_spmd
from concourse.masks import make_identity

# Problem shape (hardcoded per contract)
B, SQ, SK = 4, 2048, 2048
D_PRE = 1024
H, D_QK, D_V = 16, 64, 64
N_CORES = 8
HALF = (H // 2) * D_QK  # 512 columns of the projection handled per core
N_PAIRS = 4  # head pairs per core
S_CHUNK = 512  # moving free-dim per matmul
N_DT = D_PRE // 128  # d_pre tiles of 128
N_KT = SK // 128  # key tiles of 128
N_KTP = N_KT // 2  # key-tile pairs (DoubleRow granularity)
N_QC = SQ // S_CHUNK  # query chunks of 512
MASK_NEG = -30000.0
EX_OFF = 0.0
V_STRIDE = 130  # v pair-tile inner stride (65 cols per head)

F32 = mybir.dt.float32
BF16 = mybir.dt.bfloat16
FP8 = mybir.dt.float8e4
BF16_NP = np.dtype(ml_dtypes.bfloat16)


_COMPILED = None


def _build_program():
    nc = bacc.Bacc("TRN2", target_bir_lowering=False, debug=False)

    # DRAM I/O (names are the in_map keys)
    xq = nc.dram_tensor("xq", [D_PRE, SQ], BF16, kind="ExternalInput").ap()
    xk = nc.dram_tensor("xk", [D_PRE, SK], BF16, kind="ExternalInput").ap()
    xv = nc.dram_tensor("xv", [D_PRE, SK], BF16, kind="ExternalInput").ap()
    wq = nc.dram_tensor("wq", [D_PRE, HALF], BF16, kind="ExternalInput").ap()
    wk = nc.dram_tensor("wk", [D_PRE, HALF], BF16, kind="ExternalInput").ap()
    # v weights with a zero column appended per head (ones column generator)
    wv = nc.dram_tensor("wv", [D_PRE, N_PAIRS * 130], BF16, kind="ExternalInput").ap()
    bq = nc.dram_tensor("bq", [128, N_PAIRS], F32, kind="ExternalInput").ap()
    bk = nc.dram_tensor("bk", [128, N_PAIRS], F32, kind="ExternalInput").ap()
    bv = nc.dram_tensor("bv", [128, N_PAIRS * 130], F32, kind="ExternalInput").ap()
    mb = nc.dram_tensor("mb", [128, N_KT], F32, kind="ExternalInput").ap()
    out = nc.dram_tensor("out", [SQ, HALF], F32, kind="ExternalOutput").ap()

    with tile.TileContext(nc) as tc:
        _emit(tc, xq, xk, xv, wq, wk, wv, bq, bk, bv, mb, out)

    nc.compile()
    return nc


def _emit(tc, xq, xk, xv, wq, wk, wv, bq, bk, bv, mb, out):
    nc = tc.nc

    with ExitStack() as ctx:
        # ---- pools ----
        xp = ctx.enter_context(tc.tile_pool(name="x", bufs=3 * N_DT * N_QC))
        wp = ctx.enter_context(tc.tile_pool(name="w", bufs=1))
        cp = ctx.enter_context(tc.tile_pool(name="const", bufs=1))
        qkvp = ctx.enter_context(tc.tile_pool(name="qkv", bufs=1))
        expp = ctx.enter_context(tc.tile_pool(name="exp", bufs=11))
        avtp = ctx.enter_context(tc.tile_pool(name="avt", bufs=2))
        stgp = ctx.enter_context(tc.tile_pool(name="stg", bufs=4))
        rp = ctx.enter_context(tc.tile_pool(name="recip", bufs=8))

        proj_ps = ctx.enter_context(tc.tile_pool(name="proj_ps", bufs=1, space="PSUM"))
        sc_ps = ctx.enter_context(tc.tile_pool(name="sc_ps", bufs=2, space="PSUM"))
        av_ps = ctx.enter_context(tc.tile_pool(name="av_ps", bufs=2, space="PSUM"))
        tp_ps = ctx.enter_context(tc.tile_pool(name="tp_ps", bufs=1, space="PSUM"))

        # ---- constants (tiny, load first) ----
        ident = cp.tile([128, 128], F32, name="ident")
        make_identity(nc, ident)
        mb_sb = cp.tile([128, N_KT], F32, name="mb_sb")
        nc.sync.dma_start(mb_sb, mb)
        bq_sb = cp.tile([128, N_PAIRS], F32, name="bq_sb")
        nc.sync.dma_start(bq_sb, bq)
        bk_sb = cp.tile([128, N_PAIRS], F32, name="bk_sb")
        nc.sync.dma_start(bk_sb, bk)
        bv_sb = cp.tile([128, N_PAIRS * 130], F32, name="bv_sb")
        nc.sync.dma_start(bv_sb, bv)

        # ---- streamed loads in qc-chunks, ordered so the attention stream
        # unblocks ASAP: wq,wk + c0 of xq,xk first; xk tail (kt sweep), then
        # xq tail (later qc), then xv + wv (AV is deferred past these) ----
        def load_w(wap, pfx, width):
            ts = []
            for dt_i in range(N_DT):
                t = wp.tile([128, width], BF16, name=f"{pfx}{dt_i}", tag=f"{pfx}{dt_i}")
                nc.sync.dma_start(t, wap[dt_i * 128 : (dt_i + 1) * 128, :])
                ts.append(t)
            return ts

        x_tiles = {}  # (pfx, dt, c) -> [128, 512] bf16

        def load_x_chunk(xap, pfx, c):
            for dt_i in range(N_DT):
                t = xp.tile([128, S_CHUNK], BF16, name=f"{pfx}{dt_i}_{c}", tag="x")
                nc.sync.dma_start(
                    t, xap[dt_i * 128 : (dt_i + 1) * 128, c * S_CHUNK : (c + 1) * S_CHUNK]
                )
                x_tiles[(pfx, dt_i, c)] = t

        wq_sb = load_w(wq, "wq", HALF)
        wk_sb = load_w(wk, "wk", HALF)
        load_x_chunk(xq, "q", 0)
        load_x_chunk(xk, "k", 0)
        load_x_chunk(xk, "k", 1)
        wv_sb = load_w(wv, "wv", N_PAIRS * 130)
        load_x_chunk(xv, "v", 0)
        load_x_chunk(xk, "k", 2)
        load_x_chunk(xv, "v", 1)
        load_x_chunk(xk, "k", 3)
        load_x_chunk(xq, "q", 1)
        load_x_chunk(xv, "v", 2)
        load_x_chunk(xq, "q", 2)
        load_x_chunk(xv, "v", 3)
        load_x_chunk(xq, "q", 3)

        v_tiles = {}  # (pair, ktp) -> [128, 2, V_STRIDE] fp8 tile
        qkT = {}  # (pfx, pair) -> [128, SQ] bf16 tile

        def qk_tile(pfx, pair):
            if (pfx, pair) not in qkT:
                qkT[(pfx, pair)] = qkvp.tile(
                    [128, SQ], BF16, name=f"{pfx}T{pair}", tag=f"{pfx}T", bufs=2
                )
            return qkT[(pfx, pair)]

        def v_tile(pair, ktp):
            if (pair, ktp) not in v_tiles:
                v_tiles[(pair, ktp)] = qkvp.tile(
                    [128, 2 * V_STRIDE], BF16, name=f"v{pair}_{ktp}", tag="v",
                    bufs=N_PAIRS * N_KTP,
                )
            return v_tiles[(pair, ktp)]

        proj_ps_open = {}

        def emit_qk_half(pair, pfx, qc, half):
            # half a [128, 512] projection chunk: 4 accumulating MMs; the
            # second half adds the bias copy. Halves of one chunk must be
            # popped back-to-back (proj_ps bufs=1).
            dst = qk_tile(pfx, pair)
            w_sb = wq_sb if pfx == "q" else wk_sb
            b_sb = bq_sb if pfx == "q" else bk_sb
            xpfx = "q" if pfx == "q" else "k"
            key = (pair, pfx, qc)
            if half == 1:
                ps = proj_ps_open.pop(key)
            else:
                ps = proj_ps.tile(
                    [128, S_CHUNK], F32, name=f"{pfx}ps{pair}_{qc}", tag="proj"
                )
            for dt_i in range(half * 4, half * 4 + 4):
                nc.tensor.matmul(
                    ps,
                    lhsT=w_sb[dt_i][:, pair * 128 : (pair + 1) * 128],
                    rhs=x_tiles[(xpfx, dt_i, qc)],
                    start=(dt_i == 0),
                    stop=(dt_i == N_DT - 1),
                )
            if half == 0:
                proj_ps_open[key] = ps
            else:
                nc.vector.tensor_scalar_add(
                    dst[:, qc * S_CHUNK : (qc + 1) * S_CHUNK],
                    ps,
                    b_sb[:, pair : pair + 1],
                )

        def emit_v_half(g, st, half):
            # v projection for pairs (2g, 2g+1), one key tile st, one half of
            # the contraction; half 1 finishes with fp8 writes into the
            # DoubleRow pair-tiles.
            key = ("v", g, st)
            if half == 1:
                ps = proj_ps_open.pop(key)
            else:
                ps = proj_ps.tile([128, S_CHUNK], F32, name=f"vps{g}_{st}", tag="proj")
            qc = st // 4  # key tile st lives in x chunk st//4
            col = (st % 4) * 128
            for dt_i in range(half * 4, half * 4 + 4):
                nc.tensor.matmul(
                    ps[:, 0:260],
                    lhsT=x_tiles[("v", dt_i, qc)][:, col : col + 128],
                    rhs=wv_sb[dt_i][:, g * 260 : (g + 1) * 260],
                    start=(dt_i == 0),
                    stop=(dt_i == N_DT - 1),
                )
            if half == 0:
                proj_ps_open[key] = ps
                return
            ktp, j = st // 2, st % 2
            for p_i in range(2):
                pair = 2 * g + p_i
                vt = v_tile(pair, ktp)
                nc.vector.tensor_add(
                    vt[:, j * 130 : j * 130 + 130],
                    ps[:, p_i * 130 : p_i * 130 + 130],
                    bv_sb[:, pair * 130 : pair * 130 + 130],
                )

        # filler queue: 4-MM units popped POPS_PER_BLOCK per block. Emission
        # order IS program order per engine — every tile write must be
        # emitted before its readers. Gates below are derived from unit
        # positions. Halves of a chunk stay adjacent (shared open psum).
        filler = []

        def _qk(pair, pfx, qc, half):
            return lambda: emit_qk_half(pair, pfx, qc, half)

        def _v(g, st, half):
            return lambda: emit_v_half(g, st, half)

        v_gate = {}  # (g, st) -> first block the v pair-half is available

        # pop schedule: 3/block for blocks 0-14 (v g0 + p0 tail), then 1/block
        def pops_at(b):
            return 3 if b < 15 else 1

        def pop_block(pos):
            if pos < 45:
                return pos // 3
            return 15 + (pos - 45)

        def add_v(g, st, half):
            filler.append(_v(g, st, half))
            if half == 1:
                v_gate[(g, st)] = pop_block(len(filler) - 1) + 1

        # interleaved head: p0 k-tail (kt sweep gates) and v g0 (AV gates),
        # then p0 q-tail, p1, v g1 interleaved with p2, p3. Popped 3/block
        # for blocks 0-14 (pos 0-44), then 1/block (pos 45+k -> block 15+k).
        qk_gate = {}

        def add_qk(pair, pfx, c, half):
            filler.append(_qk(pair, pfx, c, half))
            if half == 1:
                qk_gate[(pair, pfx, c)] = pop_block(len(filler) - 1) + 1

        add_qk(0, "k", 1, 0); add_qk(0, "k", 1, 1)       # pos 0-1
        add_v(0, 0, 0); add_v(0, 0, 1)                   # 2-3
        add_qk(0, "k", 2, 0); add_qk(0, "k", 2, 1)       # 4-5
        add_v(0, 1, 0); add_v(0, 1, 1)                   # 6-7
        add_qk(0, "k", 3, 0); add_qk(0, "k", 3, 1)       # 8-9
        for st in (2, 3):
            add_v(0, st, 0); add_v(0, st, 1)             # 10-13
        add_qk(0, "q", 1, 0); add_qk(0, "q", 1, 1)       # 14-15
        for st in (4, 5, 6, 7):
            add_v(0, st, 0); add_v(0, st, 1)             # 16-23
        add_qk(0, "q", 2, 0); add_qk(0, "q", 2, 1)       # 24-25
        for st in (8, 9, 10, 11):
            add_v(0, st, 0); add_v(0, st, 1)             # 26-33
        add_qk(0, "q", 3, 0); add_qk(0, "q", 3, 1)       # 34-35
        for st in (12, 13, 14, 15):
            add_v(0, st, 0); add_v(0, st, 1)             # 36-43
        add_qk(1, "q", 0, 0)                             # 44 (front ends)
        add_qk(1, "q", 0, 1)                             # 45
        add_qk(1, "k", 0, 0); add_qk(1, "k", 0, 1)       # 46-47
        for pfx in ("q", "k"):
            for c in range(1, N_QC):
                add_qk(1, pfx, c, 0); add_qk(1, pfx, c, 1)  # 48-59
        for st in range(14):
            add_v(1, st, 0); add_v(1, st, 1)             # 60-87
        add_qk(2, "q", 0, 0); add_qk(2, "q", 0, 1)       # 88-89
        add_qk(2, "k", 0, 0); add_qk(2, "k", 0, 1)       # 90-91
        add_qk(2, "k", 1, 0); add_qk(2, "k", 1, 1)       # 92-93
        add_qk(2, "k", 2, 0); add_qk(2, "k", 2, 1)       # 94-95
        add_qk(2, "k", 3, 0); add_qk(2, "k", 3, 1)       # 96-97
        add_qk(2, "q", 1, 0); add_qk(2, "q", 1, 1)       # 98-99
        add_v(1, 14, 0); add_v(1, 14, 1)                 # 100-101
        add_v(1, 15, 0); add_v(1, 15, 1)                 # 102-103
        add_qk(2, "q", 2, 0); add_qk(2, "q", 2, 1)       # 104-105
        add_qk(2, "q", 3, 0); add_qk(2, "q", 3, 1)       # 106-107
        for pfx in ("q", "k"):
            for c in range(N_QC):
                add_qk(3, pfx, c, 0); add_qk(3, pfx, c, 1)  # 108-123

        def pop_filler(n):
            for _ in range(n):
                if filler:
                    filler.pop(0)()

        # warm the ACT exp table during the input DMA
        warm = cp.tile([128, 1], F32, name="actwarm")
        nc.scalar.activation(warm, bq_sb[:, 0:1],
                             mybir.ActivationFunctionType.Exp, bias=0.0, scale=0.0)

        # prologue: only what gates the first scores tiles
        emit_qk_half(0, "q", 0, 0)
        emit_qk_half(0, "q", 0, 1)
        emit_qk_half(0, "k", 0, 0)
        emit_qk_half(0, "k", 0, 1)

        # ---- software-pipelined attention stream over (pair, qc, kt) ----
        iters = [
            (pair, qc, kt)
            for pair in range(N_PAIRS)
            for qc in range(N_QC)
            for kt in range(N_KT)
        ]
        n_it = len(iters)
        sc_map = {}
        av_map = {}
        ex_map = {}  # block -> expair tile [128, 2, 1024] fp8

        def emit_scores(i, blk=None):
            pair, qc, kt = iters[i]
            if blk is not None and pair > 0:
                assert qk_gate[(pair, "q", qc)] <= blk, (i, "q", qc)
                assert qk_gate[(pair, "k", kt // 4)] <= blk, (i, "k", kt // 4)
            qT = qk_tile("q", pair)
            kT = qk_tile("k", pair)
            sc = sc_ps.tile([128, 1024], F32, name=f"sc{pair}_{qc}_{kt}", tag="sc")
            # scoresT for heads A and B, row-tiled concurrently (K=64 groups)
            nc.tensor.matmul(
                sc[:, 0:512],
                lhsT=kT[0:64, kt * 128 : (kt + 1) * 128],
                rhs=qT[0:64, qc * S_CHUNK : (qc + 1) * S_CHUNK],
                start=True,
                stop=True,
            )
            nc.tensor.matmul(
                sc[:, 512:1024],
                lhsT=kT[64:128, kt * 128 : (kt + 1) * 128],
                rhs=qT[64:128, qc * S_CHUNK : (qc + 1) * S_CHUNK],
                start=True,
                stop=True,
            )
            sc_map[i] = sc

        def emit_exp(i, ex):
            pair, qc, kt = iters[i]
            nc.scalar.activation(
                ex[:, (kt % 2) * 1024 : (kt % 2) * 1024 + 1024],
                sc_map.pop(i),
                mybir.ActivationFunctionType.Exp,
                bias=mb_sb[:, kt : kt + 1],
                scale=0.125,
            )

        epi_pending = []

        def emit_epilogue_half(pair, qc, avts, stgs, h_i):
            # transpose one head back to [q, d_v], normalize; store when done.
            # Halves run in consecutive blocks to halve the per-block PE spike.
            if stgs is None:
                stgs = [
                    stgp.tile([128, 128], F32, name=f"st{pair}_{qc}_{u}", tag="stg")
                    for u in range(4)
                ]
            avt = avts[h_i]
            tp = tp_ps.tile([128, 260], F32, name=f"tp{pair}_{qc}_{h_i}", tag="tp")
            for u in range(4):
                nc.tensor.transpose(
                    tp[:, u * 65 : u * 65 + 65],
                    avt[:, u * 128 : (u + 1) * 128],
                    ident[0:65, 0:65],
                )
            for u in range(4):
                rc = rp.tile([128, 1], F32, name=f"rc{pair}_{qc}_{h_i}_{u}", tag="rc")
                nc.vector.reciprocal(rc, tp[:, u * 65 + 64 : u * 65 + 65])
                nc.vector.tensor_scalar_mul(
                    stgs[u][:, h_i * 64 : (h_i + 1) * 64],
                    tp[:, u * 65 : u * 65 + 64],
                    rc,
                )
            if h_i == 1:
                for u in range(4):
                    qt = qc * 4 + u
                    nc.sync.dma_start(
                        out[qt * 128 : (qt + 1) * 128, pair * 128 : (pair + 1) * 128],
                        stgs[u],
                    )
            return stgs

        av_active = []  # (pair, qc) sets with an open av psum pair (max 2)

        def emit_av_block(b):
            # DoubleRow AV for block b = iters (2b, 2b+1): 2 MMs, both heads
            pair, qc, kt = iters[2 * b]
            ktp = kt // 2
            ex = ex_map.pop(b)
            if ktp == 0:
                av_map[(pair, qc)] = (
                    av_ps.tile([65, S_CHUNK], F32, name=f"ava{pair}_{qc}", tag="av"),
                    av_ps.tile([65, S_CHUNK], F32, name=f"avb{pair}_{qc}", tag="av"),
                )
                av_active.append((pair, qc))
            av_a, av_b = av_map[(pair, qc)]
            assert (pair, ktp) in v_tiles, (pair, ktp)
            vt = v_tiles[(pair, ktp)]
            for j in range(2):
                st_ = ktp == 0 and j == 0
                sp_ = ktp == N_KTP - 1 and j == 1
                vo, eo = j * 130, j * 1024
                nc.tensor.matmul(
                    av_a, lhsT=vt[:, vo : vo + 65],
                    rhs=ex[:, eo : eo + 512],
                    start=st_, stop=sp_,
                )
                nc.tensor.matmul(
                    av_b, lhsT=vt[:, vo + 65 : vo + 130],
                    rhs=ex[:, eo + 512 : eo + 1024],
                    start=st_, stop=sp_,
                )
            if ktp == N_KTP - 1:
                av_a, av_b = av_map.pop((pair, qc))
                avts = []
                for h_i, av in enumerate((av_a, av_b)):
                    avt = avtp.tile(
                        [65, S_CHUNK], F32, name=f"avt{pair}_{qc}_{h_i}", tag="avt"
                    )
                    nc.vector.tensor_copy(avt, av)
                    avts.append(avt)
                epi_pending.append((pair, qc, avts, None, 0))
                av_active.remove((pair, qc))

        def av_legal(b_pend, b_now):
            # emission-order gates: v pair-tile written, expair written (a
            # previous block), and at most 2 av psum sets in flight
            pair, qc, kt = iters[2 * b_pend]
            ktp = kt // 2
            if v_gate[(pair // 2, 2 * ktp + 1)] > b_now:
                return False
            if b_pend >= b_now:
                return False
            if ktp == 0 and len(av_active) >= 2 and (pair, qc) not in av_active:
                return False
            return True

        # stream prologue: scores for iters 0,1 (lookahead is 2)
        emit_scores(0)
        emit_scores(1)

        n_blocks = n_it // 2
        av_pending = list(range(n_blocks))  # in-order AV emission queue
        for b in range(n_blocks):
            i0, i1 = 2 * b, 2 * b + 1
            ex = expp.tile([128, 2048], BF16, name=f"ex{b}", tag="ex")
            ex_map[b] = ex
            emit_exp(i0, ex)
            emit_exp(i1, ex)
            if i0 + 2 < n_it:
                emit_scores(i0 + 2, blk=b)
                emit_scores(i0 + 3, blk=b)
            if epi_pending:
                pair_, qc_, avts_, stgs_, h_ = epi_pending.pop(0)
                stgs_ = emit_epilogue_half(pair_, qc_, avts_, stgs_, h_)
                if h_ == 0:
                    epi_pending.insert(0, (pair_, qc_, avts_, stgs_, 1))
            budget = 3
            while av_pending and budget and av_legal(av_pending[0], b):
                emit_av_block(av_pending.pop(0))
                budget -= 1
            pop_filler(pops_at(b))
        for b_pend in av_pending:
            emit_av_block(b_pend)
        while epi_pending:
            pair_, qc_, avts_, stgs_, h_ = epi_pending.pop(0)
            stgs_ = emit_epilogue_half(pair_, qc_, avts_, stgs_, h_)
            if h_ == 0:
                epi_pending.insert(0, (pair_, qc_, avts_, stgs_, 1))

        assert not filler, f"{len(filler)} filler chunks left unscheduled"
        assert not ex_map and not av_map and not sc_map, (
            len(ex_map), len(av_map), len(sc_map))
        assert not proj_ps_open


def _prep_core_inputs(pre_qs, pre_ks, pre_vs, k_mask, q_w, q_b, k_w, k_b, v_w, v_b, core):
    b = core // 2
    hh = core % 2
    cols = slice(HALF * hh, HALF * (hh + 1))

    xq = np.ascontiguousarray(pre_qs[b].T).astype(BF16_NP)
    xk = np.ascontiguousarray(pre_ks[b].T).astype(BF16_NP)
    xv = np.ascontiguousarray(pre_vs[b].T).astype(BF16_NP)
    wq = np.ascontiguousarray(q_w[:, cols]).astype(BF16_NP)
    wk = np.ascontiguousarray(k_w[:, cols]).astype(BF16_NP)

    wv_core = v_w[:, cols].astype(np.float32)
    wv = np.zeros((D_PRE, N_PAIRS * 130), dtype=np.float32)
    bv_core = v_b[cols].astype(np.float32)
    bv_ext = np.zeros(N_PAIRS * 130, dtype=np.float32)
    for p in range(N_PAIRS):
        wv[:, p * 130 : p * 130 + 64] = wv_core[:, p * 128 : p * 128 + 64]
        wv[:, p * 130 + 65 : p * 130 + 129] = wv_core[:, p * 128 + 64 : p * 128 + 128]
        bv_ext[p * 130 : p * 130 + 64] = bv_core[p * 128 : p * 128 + 64]
        bv_ext[p * 130 + 64] = 1.0
        bv_ext[p * 130 + 65 : p * 130 + 129] = bv_core[p * 128 + 64 : p * 128 + 128]
        bv_ext[p * 130 + 129] = 1.0

    bq = np.ascontiguousarray(q_b[cols].astype(np.float32).reshape(N_PAIRS, 128).T)
    bk = np.ascontiguousarray(k_b[cols].astype(np.float32).reshape(N_PAIRS, 128).T)
    bv_full = np.ascontiguousarray(np.tile(bv_ext[None, :], (128, 1)))

    # mask True -> 0.0, False -> MASK_NEG; plus the fp8-range exp offset
    mbias = np.where(k_mask[b], 0.0, MASK_NEG).astype(np.float32) + EX_OFF
    mb = np.ascontiguousarray(mbias.reshape(N_KT, 128).T)

    return {
        "xq": xq,
        "xk": xk,
        "xv": xv,
        "wq": wq,
        "wk": wk,
        "wv": wv.astype(BF16_NP),
        "bq": bq,
        "bk": bk,
        "bv": bv_full,
        "mb": mb,
    }


def kernel(pre_qs, pre_ks, pre_vs, k_mask, q_w, q_b, k_w, k_b, v_w, v_b):
    global _COMPILED
    args = (pre_qs, pre_ks, pre_vs, k_mask, q_w, q_b, k_w, k_b, v_w, v_b)
    args = tuple(np.asarray(a) for a in args)

    if _COMPILED is None:
        _COMPILED = _build_program()
    nc = _COMPILED

    in_maps = [_prep_core_inputs(*args, core=c) for c in range(N_CORES)]

    trace = bool(int(os.environ.get("BASS_KERNEL_TRACE", "0")))
    res = run_bass_kernel_spmd(
        nc,
        in_maps,
        core_ids=list(range(N_CORES)),
        trace=trace,
    )
    if trace:
        kernel.last_results = res

    out = np.empty((B, SQ, H * D_V), dtype=np.float32)
    for c in range(N_CORES):
        b = c // 2
        hh = c % 2
        out[b, :, HALF * hh : HALF * (hh + 1)] = res.results[c]["out"]
    return out
